# revision 15
# baseline (speedup 1.0000x reference)
"""Trainium2 Bass kernel for dense-transformer attention block.

Problem shapes (hardcoded): x [2, 2048, 4096], 32 q-heads / 8 kv-heads,
head_dim 128, RoPE + per-head RMSNorm on q/k, causal GQA attention, out proj.

Distribution: 8 NeuronCores, DP over batch (2) x TP over heads (4).
Core c: batch c//4, q-heads [8g, 8g+8), kv-heads [2g, 2g+2), g = c%4.
Each core computes a partial [2048, 4096] output (row-parallel wo);
the host sums the 4 partials per batch element.  No collectives.
"""

import math
import sys

for _p in ("/opt/trn_rl_repo", "/opt/pypackages"):
    if _p not in sys.path:
        sys.path.append(_p)

import numpy as np

import concourse.bacc as bacc
import concourse.mybir as mybir
import concourse.tile as tile
from concourse import bass_utils
from concourse.masks import make_identity

# ---- problem constants ----
BSZ, SEQ, DIM = 2, 2048, 4096
N_HEADS, N_KV = 32, 8
HD = 128                      # head dim
EPS = 1e-5
SCALE = 1.0 / math.sqrt(HD)

# ---- per-core constants ----
NCORES = 8
GROUPS = 4                    # TP groups per batch
P = 128                       # partitions
S = SEQ                       # tokens per core (one batch element)
D = DIM
HQ = N_HEADS // GROUPS        # 8 q heads per core
HKV = N_KV // GROUPS          # 2 kv heads per core
HQH = HQ // 2                 # 4 q heads per half-pass
FQ = HQ * HD                  # 1024
FKV = HKV * HD                # 256
TB = S // P                   # 16 token blocks
KO = D // P                   # 32 contraction tiles
QC = S // 512                 # 4 query chunks
KB = S // P                   # 16 key blocks

F32 = mybir.dt.float32
F32R = mybir.dt.float32r


def _rope_rms(nc, src, dst, tmp_pool, st_pool, cos_b, sin_b, eps_sb, nh):
    """RoPE then per-head RMSNorm.  src: [P, nh*HD] (PSUM ok), dst: [P, nh*HD] SBUF."""
    se = src.rearrange("p (h j two) -> p h j two", h=nh, two=2)
    de = dst.rearrange("p (h j two) -> p h j two", h=nh, two=2)
    rt = tmp_pool.tile([P, nh * HD // 2], F32, tag="rope")
    rtv = rt.rearrange("p (h j) -> p h j", h=nh)
    # e' = e*c - o*s ; o' = e*s + o*c
    nc.vector.tensor_mul(rtv, se[..., 1], sin_b)              # o*s
    nc.vector.tensor_mul(de[..., 0], se[..., 0], cos_b)       # e*c
    nc.vector.tensor_sub(de[..., 0], de[..., 0], rtv)
    nc.vector.tensor_mul(rtv, se[..., 1], cos_b)              # o*c
    nc.vector.tensor_mul(de[..., 1], se[..., 0], sin_b)       # e*s
    nc.vector.tensor_add(de[..., 1], de[..., 1], rtv)
    # RMSNorm over each head's HD elements (Square + free accumulation on ACT)
    ssum = st_pool.tile([P, nh], F32, tag="ssum")
    for h in range(nh):
        sqd = tmp_pool.tile([P, HD], F32, tag="sqd")
        nc.scalar.activation(sqd, dst[:, h * HD:(h + 1) * HD],
                             mybir.ActivationFunctionType.Square,
                             accum_out=ssum[:, h:h + 1])
    rstd = st_pool.tile([P, nh], F32, tag="rstd")
    nc.scalar.activation(rstd, ssum, mybir.ActivationFunctionType.Sqrt,
                         bias=eps_sb, scale=1.0 / HD)
    nc.vector.reciprocal(rstd, rstd)
    dv = dst.rearrange("p (h d) -> p h d", h=nh)
    nc.vector.tensor_mul(dv, dv, rstd[:, :, None].to_broadcast([P, nh, HD]))


def _build(mode):
    """mode: 'causal' | 'zero' | 'general'"""
    nc = bacc.Bacc("TRN2", target_bir_lowering=False, debug=False)

    xT = nc.dram_tensor("xT", [D, S], F32R, kind="ExternalInput").ap()
    wqT = nc.dram_tensor("wqT", [D, FQ], F32R, kind="ExternalInput").ap()
    wkT = nc.dram_tensor("wkT", [D, FKV], F32R, kind="ExternalInput").ap()
    wvT = nc.dram_tensor("wvT", [D, FKV], F32R, kind="ExternalInput").ap()
    woT = nc.dram_tensor("woT", [FQ, D], F32R, kind="ExternalInput").ap()
    cosd = nc.dram_tensor("cosd", [S, HD // 2], F32, kind="ExternalInput").ap()
    sind = nc.dram_tensor("sind", [S, HD // 2], F32, kind="ExternalInput").ap()
    onesd = nc.dram_tensor("onesd", [P, 1], F32R, kind="ExternalInput").ap()
    if mode == "causal":
        dmaskd = nc.dram_tensor("dmask", [P, P], F32, kind="ExternalInput").ap()
    if mode == "general":
        maskTd = nc.dram_tensor("maskT", [S, S], F32, kind="ExternalInput").ap()
    out_p = nc.dram_tensor("out_p", [S, D], F32, kind="ExternalOutput").ap()

    xTr = xT.rearrange("(ko p) s -> p ko s", p=P)
    wqTr = wqT.rearrange("(ko p) f -> p ko f", p=P)
    wkTr = wkT.rearrange("(ko p) f -> p ko f", p=P)
    wvTr = wvT.rearrange("(ko p) f -> p ko f", p=P)
    woTr = woT.rearrange("(h p) d -> p h d", p=P)

    Exp = mybir.ActivationFunctionType.Exp

    with tile.TileContext(nc) as tc:
        singles = tc.alloc_tile_pool(name="singles", bufs=1)
        dramp = tc.alloc_tile_pool(name="dramp", bufs=1, space="DRAM")
        psum = tc.alloc_tile_pool(name="psum", bufs=1, space="PSUM")
        p_ot = tc.alloc_tile_pool(name="p_ot", bufs=3)
        p_w = tc.alloc_tile_pool(name="p_w", bufs=1)
        kvp = tc.alloc_tile_pool(name="kvp", bufs=1)
        p_x = tc.alloc_tile_pool(name="p_x", bufs=2)
        p_tmp = tc.alloc_tile_pool(name="p_tmp", bufs=3)
        p_st = tc.alloc_tile_pool(name="p_st", bufs=4)
        p_qt = tc.alloc_tile_pool(name="p_qt", bufs=1)
        p_pb = tc.alloc_tile_pool(name="p_pb", bufs=3)

        cos_sb = singles.tile([P, TB, HD // 2], F32)
        nc.sync.dma_start(cos_sb, cosd.rearrange("(n p) c -> p n c", p=P))
        sin_sb = singles.tile([P, TB, HD // 2], F32)
        nc.sync.dma_start(sin_sb, sind.rearrange("(n p) c -> p n c", p=P))
        ones_sb = singles.tile([P, 1], F32R)
        nc.sync.dma_start(ones_sb, onesd)
        eps_sb = singles.tile([P, 1], F32)
        nc.vector.memset(eps_sb, EPS)
        ident = singles.tile([P, P], F32)
        make_identity(nc, ident)
        if mode == "causal":
            dmask_sb = singles.tile([P, P], F32)
            nc.sync.dma_start(dmask_sb, dmaskd)

        oTd = dramp.tile([HQ, P, S], F32R)   # attention output^T staging (DRAM)

        # ---------------- Phase KV: K/V projection + rope/rms + K transpose --
        KT = kvp.tile([P, HKV, S], F32R)      # [hd, kvh, tok]
        V = kvp.tile([P, TB, FKV], F32R)      # [tok%P, tb, kvh*HD]
        wkv = p_w.tile([P, KO, 2 * FKV], F32R, tag="w")
        nc.sync.dma_start(wkv[:, :, 0:FKV], wkTr)
        nc.sync.dma_start(wkv[:, :, FKV:], wvTr)
        for tb in range(TB):
            xt = p_x.tile([P, KO, P], F32R, tag="x")
            nc.sync.dma_start(xt, xTr[:, :, tb * P:(tb + 1) * P])
            pkv = psum.tile([P, 2 * FKV], F32, tag="mm", bufs=4)
            for ko in range(KO):
                nc.tensor.matmul(pkv, xt[:, ko, :], wkv[:, ko, :],
                                 start=(ko == 0), stop=(ko == KO - 1))
            nc.scalar.copy(V[:, tb, :], pkv[:, FKV:])
            ktmp = p_tmp.tile([P, FKV], F32, tag="qk")
            cos_b = cos_sb[:, tb, None, :].to_broadcast([P, HKV, HD // 2])
            sin_b = sin_sb[:, tb, None, :].to_broadcast([P, HKV, HD // 2])
            _rope_rms(nc, pkv[:, 0:FKV], ktmp, p_tmp, p_st, cos_b, sin_b, eps_sb, HKV)
            for h in range(HKV):
                ptr = psum.tile([P, P], F32, tag="small", bufs=2)
                nc.tensor.transpose(ptr, ktmp[:, h * HD:(h + 1) * HD], ident)
                nc.scalar.copy(KT[:, h, tb * P:(tb + 1) * P], ptr)

        # ---------------- Q halves + attention ------------------------------
        for half in range(2):
            q0 = half * HQH
            QT = p_qt.tile([P, HQH, S], F32R, tag="qt")   # [hd, h, tok]
            wq = p_w.tile([P, KO, 2 * FKV], F32R, tag="w")
            nc.sync.dma_start(wq, wqTr[:, :, q0 * HD:(q0 + HQH) * HD])
            for tb in range(TB):
                xt = p_x.tile([P, KO, P], F32R, tag="x")
                nc.sync.dma_start(xt, xTr[:, :, tb * P:(tb + 1) * P])
                pq = psum.tile([P, HQH * HD], F32, tag="mm", bufs=4)
                for ko in range(KO):
                    nc.tensor.matmul(pq, xt[:, ko, :], wq[:, ko, :],
                                     start=(ko == 0), stop=(ko == KO - 1))
                qtmp = p_tmp.tile([P, HQH * HD], F32, tag="qk")
                cos_b = cos_sb[:, tb, None, :].to_broadcast([P, HQH, HD // 2])
                sin_b = sin_sb[:, tb, None, :].to_broadcast([P, HQH, HD // 2])
                _rope_rms(nc, pq, qtmp, p_tmp, p_st, cos_b, sin_b, eps_sb, HQH)
                for h in range(HQH):
                    ptr = psum.tile([P, P], F32, tag="small", bufs=2)
                    nc.tensor.transpose(ptr, qtmp[:, h * HD:(h + 1) * HD], ident)
                    nc.scalar.copy(QT[:, h, tb * P:(tb + 1) * P], ptr)

            if half == 1:
                # prefetch wo rows for heads 0-3 into the freed weight slot;
                # the DMA overlaps attention of the second half
                woT0 = p_w.tile([P, KO, 2 * FKV], F32R, tag="w")
                nc.sync.dma_start(
                    woT0.rearrange("p ko f -> p (ko f)").rearrange(
                        "p (h d) -> p h d", h=HQH),
                    woTr[:, 0:HQH, :])

            # attention for these 4 heads; kv head == half
            kvh = half
            for h in range(HQH):
                for qc in range(QC):
                    kb_hi = 4 * (qc + 1) if mode == "causal" else KB
                    po = psum.tile([P, 512], F32, tag="po", bufs=2)
                    pd = psum.tile([1, 512], F32, tag="small", bufs=2)
                    for kb in range(kb_hi):
                        j0 = max(0, kb * P - qc * 512) if mode == "causal" else 0
                        pss = psum.tile([P, 512], F32, tag="mm", bufs=4)
                        nc.tensor.matmul(
                            pss[:, j0:], KT[:, kvh, kb * P:(kb + 1) * P],
                            QT[:, h, qc * 512 + j0:(qc + 1) * 512],
                            start=True, stop=True)
                        if mode == "general":
                            mt = p_pb.tile([P, 512], F32, tag="mt")
                            nc.sync.dma_start(
                                mt, maskTd[kb * P:(kb + 1) * P, qc * 512:(qc + 1) * 512])
                            nc.vector.tensor_add(pss, pss, mt)
                        pbt = p_pb.tile([P, 512], F32R, tag="pb")
                        nc.scalar.activation(pbt[:, j0:], pss[:, j0:], Exp, scale=SCALE)
                        if mode == "causal" and kb >= 4 * qc:
                            nc.vector.tensor_mul(pbt[:, j0:j0 + P],
                                                 pbt[:, j0:j0 + P], dmask_sb)
                        first, last = (kb == 0), (kb == kb_hi - 1)
                        nc.tensor.matmul(po[:, j0:],
                                         V[:, kb, kvh * HD:(kvh + 1) * HD],
                                         pbt[:, j0:], start=first, stop=last)
                        nc.tensor.matmul(pd[:, j0:], ones_sb, pbt[:, j0:],
                                         start=first, stop=last)
                    rec = p_st.tile([1, 512], F32, tag="rec", bufs=2)
                    nc.vector.reciprocal(rec, pd)
                    recb = p_pb.tile([P, 512], F32, tag="recb", bufs=2)
                    nc.gpsimd.partition_broadcast(recb, rec)
                    ot = p_ot.tile([P, 512], F32R, tag="ot")
                    nc.vector.tensor_mul(ot, po, recb)
                    nc.sync.dma_start(oTd[q0 + h, :, qc * 512:(qc + 1) * 512], ot)

        p_pb.release()
        p_qt.release()
        p_st.release()
        p_tmp.release()
        p_x.release()
        kvp.release()

        # ---------------- O projection --------------------------------------
        p_wo = tc.alloc_tile_pool(name="p_wo", bufs=1)
        p_ox = tc.alloc_tile_pool(name="p_ox", bufs=2)
        woT1 = p_wo.tile([P, HQH, D], F32R)
        nc.sync.dma_start(woT1, woTr[:, HQH:, :])
        woT0v = woT0.rearrange("p ko f -> p (ko f)").rearrange(
            "p (h d) -> p h d", h=HQH)
        for tb in range(TB):
            oT = p_ox.tile([P, HQ, P], F32R, tag="ox")
            nc.sync.dma_start(oT, oTd[:, :, tb * P:(tb + 1) * P].rearrange("h p s -> p h s"))
            for dc in range(D // 512):
                pso = psum.tile([P, 512], F32, tag="mm", bufs=4)
                for ko in range(HQ):
                    w_sl = (woT0v[:, ko, dc * 512:(dc + 1) * 512] if ko < HQH
                            else woT1[:, ko - HQH, dc * 512:(dc + 1) * 512])
                    nc.tensor.matmul(pso, oT[:, ko, :], w_sl,
                                     start=(ko == 0), stop=(ko == HQ - 1))
                osb = p_ot.tile([P, 512], F32, tag="osb")
                nc.scalar.copy(osb, pso)
                nc.sync.dma_start(out_p[tb * P:(tb + 1) * P, dc * 512:(dc + 1) * 512], osb)
        p_ox.release()
        p_wo.release()
        p_w.release()
        p_ot.release()
        psum.release()
        dramp.release()
        singles.release()

    nc.compile()
    return nc


_NC_CACHE = {}


def _get_nc(mode):
    if mode not in _NC_CACHE:
        _NC_CACHE[mode] = _build(mode)
    return _NC_CACHE[mode]


def _detect_mode(mask):
    il, jl = np.tril_indices(SEQ)
    if np.all(mask == 0.0):
        return "zero"
    iu, ju = np.triu_indices(SEQ, k=1)
    if np.all(mask[il, jl] == 0.0) and np.all(mask[iu, ju] <= -1e30):
        return "causal"
    return "general"


def _prep_in_maps(x, wq, wk, wv, wo, cos, sin, mask, mode):
    f = np.float32
    maps = []
    cosf = np.ascontiguousarray(cos, dtype=f)
    sinf = np.ascontiguousarray(sin, dtype=f)
    if mode == "causal":
        i = np.arange(P)
        dmask = (i[:, None] <= i[None, :]).astype(f)   # keep where k <= q
    if mode == "general":
        maskT = np.maximum(mask.T.astype(np.float64) / SCALE, -3e38).astype(f)
        maskT = np.ascontiguousarray(maskT)
    for c in range(NCORES):
        b, g = divmod(c, GROUPS)
        m = {
            "xT": np.ascontiguousarray(x[b].T, dtype=f),
            "wqT": np.ascontiguousarray(wq[FQ * g:FQ * (g + 1)].T, dtype=f),
            "wkT": np.ascontiguousarray(wk[FKV * g:FKV * (g + 1)].T, dtype=f),
            "wvT": np.ascontiguousarray(wv[FKV * g:FKV * (g + 1)].T, dtype=f),
            "woT": np.ascontiguousarray(wo[:, FQ * g:FQ * (g + 1)].T, dtype=f),
            "cosd": cosf,
            "sind": sinf,
            "onesd": np.ones((P, 1), dtype=f),
        }
        if mode == "causal":
            m["dmask"] = dmask
        if mode == "general":
            m["maskT"] = maskT
        maps.append(m)
    return maps


def _numpy_fallback(x, wq, wk, wv, wo, cos, sin, mask):
    """Exact-semantics numpy fallback (only for unexpected inputs)."""
    b, s, _ = x.shape
    xq = (x @ wq.T).reshape(b, s, N_HEADS, HD)
    xk = (x @ wk.T).reshape(b, s, N_KV, HD)
    xv = (x @ wv.T).reshape(b, s, N_KV, HD)

    def rope(t):
        tf = t.reshape(*t.shape[:-1], HD // 2, 2)
        e, o = tf[..., 0], tf[..., 1]
        c = cos[None, :, None, :]
        sn = sin[None, :, None, :]
        return np.stack([e * c - o * sn, e * sn + o * c], axis=-1).reshape(t.shape)

    def rms(t):
        return t * (1.0 / np.sqrt(np.mean(t * t, axis=-1, keepdims=True) + EPS))

    xq = rms(rope(xq))
    xk = rms(rope(xk))
    q = xq.transpose(0, 2, 1, 3).reshape(b, N_KV, N_HEADS // N_KV, s, HD)
    k = xk.transpose(0, 2, 1, 3)
    v = xv.transpose(0, 2, 1, 3)
    scores = np.einsum("bgrqd,bgkd->bgrqk", q, k) * SCALE + mask[None, None, None]
    scores -= scores.max(axis=-1, keepdims=True)
    p = np.exp(scores)
    p /= p.sum(axis=-1, keepdims=True)
    out = np.einsum("bgrqk,bgkd->bgrqd", p, v)
    out = out.reshape(b, N_HEADS, s, HD).transpose(0, 2, 1, 3).reshape(b, s, N_HEADS * HD)
    return (out @ wo.T).astype(np.float32)


def kernel(x, wq, wk, wv, wo, cos, sin, mask, start_pos=0, **_unused):
    x = np.asarray(x, dtype=np.float32)
    wq = np.asarray(wq, dtype=np.float32)
    wk = np.asarray(wk, dtype=np.float32)
    wv = np.asarray(wv, dtype=np.float32)
    wo = np.asarray(wo, dtype=np.float32)
    cos = np.asarray(cos, dtype=np.float32)
    sin = np.asarray(sin, dtype=np.float32)
    mask = np.asarray(mask, dtype=np.float32)
    if int(np.asarray(start_pos)) != 0:
        return _numpy_fallback(x, wq, wk, wv, wo, cos, sin, mask)

    mode = _detect_mode(mask)
    nc = _get_nc(mode)
    in_maps = _prep_in_maps(x, wq, wk, wv, wo, cos, sin, mask, mode)
    res = bass_utils.run_bass_kernel_spmd(nc, in_maps, core_ids=list(range(NCORES)))
    out = np.empty((BSZ, S, D), dtype=np.float32)
    for b in range(BSZ):
        acc = res.results[b * GROUPS]["out_p"].astype(np.float32).copy()
        for g in range(1, GROUPS):
            acc += res.results[b * GROUPS + g]["out_p"]
        out[b] = acc
    return out


if __name__ == "__main__":
    # quick self-exercise with random inputs (not the reference distribution)
    rng = np.random.default_rng(0)
    x = rng.standard_normal((BSZ, SEQ, DIM), dtype=np.float32)
    wq = (rng.standard_normal((DIM, DIM), dtype=np.float32) * 0.02)
    wk = (rng.standard_normal((N_KV * HD, DIM), dtype=np.float32) * 0.02)
    wv = (rng.standard_normal((N_KV * HD, DIM), dtype=np.float32) * 0.02)
    wo = (rng.standard_normal((DIM, DIM), dtype=np.float32) * 0.02)
    inv = 1.0 / (500000.0 ** (np.arange(0, HD, 2) / HD))
    ang = np.arange(SEQ)[:, None] * inv[None, :]
    cos, sin = np.cos(ang).astype(np.float32), np.sin(ang).astype(np.float32)
    mask = np.where(np.tril(np.ones((SEQ, SEQ), bool)), 0.0,
                    np.finfo(np.float32).min).astype(np.float32)
    got = kernel(x=x, wq=wq, wk=wk, wv=wv, wo=wo, cos=cos, sin=sin,
                 mask=mask, start_pos=0)
    want = _numpy_fallback(x, wq, wk, wv, wo, cos, sin, mask)
    err = np.linalg.norm(got - want) / np.linalg.norm(want)
    print("rel err vs numpy:", err)


# revision 16
# speedup vs baseline: 1.2030x; 1.2030x over previous
"""Trainium2 Bass kernel for dense-transformer attention block.

Problem shapes (hardcoded): x [2, 2048, 4096], 32 q-heads / 8 kv-heads,
head_dim 128, RoPE + per-head RMSNorm on q/k, causal GQA attention, out proj.

Distribution: 8 NeuronCores, DP over batch (2) x TP over heads (4).
Core c: batch c//4, q-heads [8g, 8g+8), kv-heads [2g, 2g+2), g = c%4.
Each core computes a partial [2048, 4096] output (row-parallel wo);
the host sums the 4 partials per batch element.  No collectives.
"""

import math
import sys

for _p in ("/opt/trn_rl_repo", "/opt/pypackages"):
    if _p not in sys.path:
        sys.path.append(_p)

import numpy as np

import concourse.bacc as bacc
import concourse.mybir as mybir
import concourse.tile as tile
from concourse import bass_utils
from concourse.masks import make_identity

# ---- problem constants ----
BSZ, SEQ, DIM = 2, 2048, 4096
N_HEADS, N_KV = 32, 8
HD = 128                      # head dim
EPS = 1e-5
SCALE = 1.0 / math.sqrt(HD)

# ---- per-core constants ----
NCORES = 8
GROUPS = 4                    # TP groups per batch
P = 128                       # partitions
S = SEQ                       # tokens per core (one batch element)
D = DIM
HQ = N_HEADS // GROUPS        # 8 q heads per core
HKV = N_KV // GROUPS          # 2 kv heads per core
HQH = HQ // 2                 # 4 q heads per half-pass
FQ = HQ * HD                  # 1024
FKV = HKV * HD                # 256
TB = S // P                   # 16 token blocks
KO = D // P                   # 32 contraction tiles
QC = S // 512                 # 4 query chunks
KB = S // P                   # 16 key blocks

F32 = mybir.dt.float32
F32R = mybir.dt.float32r


def _rope_rms(nc, src, dst, tmp_pool, st_pool, cos_b, sin_b, eps_sb, nh):
    """RoPE then per-head RMSNorm.  src: [P, nh*HD] (PSUM ok), dst: [P, nh*HD] SBUF."""
    se = src.rearrange("p (h j two) -> p h j two", h=nh, two=2)
    de = dst.rearrange("p (h j two) -> p h j two", h=nh, two=2)
    rt = tmp_pool.tile([P, nh * HD // 2], F32, tag="rope")
    rtv = rt.rearrange("p (h j) -> p h j", h=nh)
    # e' = e*c - o*s ; o' = e*s + o*c
    nc.vector.tensor_mul(rtv, se[..., 1], sin_b)              # o*s
    nc.vector.tensor_mul(de[..., 0], se[..., 0], cos_b)       # e*c
    nc.vector.tensor_sub(de[..., 0], de[..., 0], rtv)
    nc.vector.tensor_mul(rtv, se[..., 1], cos_b)              # o*c
    nc.vector.tensor_mul(de[..., 1], se[..., 0], sin_b)       # e*s
    nc.vector.tensor_add(de[..., 1], de[..., 1], rtv)
    # RMSNorm over each head's HD elements (Square + free accumulation on ACT)
    ssum = st_pool.tile([P, nh], F32, tag="ssum")
    for h in range(nh):
        sqd = tmp_pool.tile([P, HD], F32, tag="sqd")
        nc.scalar.activation(sqd, dst[:, h * HD:(h + 1) * HD],
                             mybir.ActivationFunctionType.Square,
                             accum_out=ssum[:, h:h + 1])
    rstd = st_pool.tile([P, nh], F32, tag="rstd")
    nc.scalar.activation(rstd, ssum, mybir.ActivationFunctionType.Sqrt,
                         bias=eps_sb, scale=1.0 / HD)
    nc.vector.reciprocal(rstd, rstd)
    dv = dst.rearrange("p (h d) -> p h d", h=nh)
    nc.vector.tensor_mul(dv, dv, rstd[:, :, None].to_broadcast([P, nh, HD]))


def _build(mode):
    """mode: 'causal' | 'zero' | 'general'"""
    nc = bacc.Bacc("TRN2", target_bir_lowering=False, debug=False)

    # layouts are host-pre-shuffled so each partition reads >=2KB contiguous
    xT = nc.dram_tensor("xT", [TB, P, KO, P], F32R, kind="ExternalInput").ap()
    wqT = nc.dram_tensor("wqT", [P, KO, FQ], F32R, kind="ExternalInput").ap()
    wkT = nc.dram_tensor("wkT", [P, KO, FKV], F32R, kind="ExternalInput").ap()
    wvT = nc.dram_tensor("wvT", [P, KO, FKV], F32R, kind="ExternalInput").ap()
    woT = nc.dram_tensor("woT", [FQ, D], F32R, kind="ExternalInput").ap()
    cosd = nc.dram_tensor("cosd", [P, TB, HD // 2], F32, kind="ExternalInput").ap()
    sind = nc.dram_tensor("sind", [P, TB, HD // 2], F32, kind="ExternalInput").ap()
    onesd = nc.dram_tensor("onesd", [P, 1], F32R, kind="ExternalInput").ap()
    if mode == "causal":
        dmaskd = nc.dram_tensor("dmask", [P, P], F32, kind="ExternalInput").ap()
    if mode == "general":
        maskTd = nc.dram_tensor("maskT", [S, S], F32, kind="ExternalInput").ap()
    out_p = nc.dram_tensor("out_p", [S, D], F32, kind="ExternalOutput").ap()

    woTr = woT.rearrange("(h p) d -> p h d", p=P)

    Exp = mybir.ActivationFunctionType.Exp

    with tile.TileContext(nc) as tc:
        singles = tc.alloc_tile_pool(name="singles", bufs=1)
        dramp = tc.alloc_tile_pool(name="dramp", bufs=1, space="DRAM")
        psum = tc.alloc_tile_pool(name="psum", bufs=1, space="PSUM")
        p_ot = tc.alloc_tile_pool(name="p_ot", bufs=3)
        p_w = tc.alloc_tile_pool(name="p_w", bufs=1)
        kvp = tc.alloc_tile_pool(name="kvp", bufs=1)
        p_x = tc.alloc_tile_pool(name="p_x", bufs=2)
        p_tmp = tc.alloc_tile_pool(name="p_tmp", bufs=3)
        p_st = tc.alloc_tile_pool(name="p_st", bufs=4)
        p_qt = tc.alloc_tile_pool(name="p_qt", bufs=1)
        p_pb = tc.alloc_tile_pool(name="p_pb", bufs=3)

        cos_sb = singles.tile([P, TB, HD // 2], F32)
        nc.sync.dma_start(cos_sb, cosd)
        sin_sb = singles.tile([P, TB, HD // 2], F32)
        nc.sync.dma_start(sin_sb, sind)
        ones_sb = singles.tile([P, 1], F32R)
        nc.sync.dma_start(ones_sb, onesd)
        eps_sb = singles.tile([P, 1], F32)
        nc.vector.memset(eps_sb, EPS)
        ident = singles.tile([P, P], F32)
        make_identity(nc, ident)
        if mode == "causal":
            dmask_sb = singles.tile([P, P], F32)
            nc.sync.dma_start(dmask_sb, dmaskd)

        oTd = dramp.tile([P, HQ, S], F32R)   # attention output^T staging (DRAM)

        # ---------------- Phase KV: K/V projection + rope/rms + K transpose --
        KT = kvp.tile([P, HKV, S], F32R)      # [hd, kvh, tok]
        V = kvp.tile([P, TB, FKV], F32R)      # [tok%P, tb, kvh*HD]
        wkv = p_w.tile([P, KO, 2 * FKV], F32R, tag="w")
        nc.sync.dma_start(wkv[:, :, 0:FKV], wkT)
        nc.sync.dma_start(wkv[:, :, FKV:], wvT)
        for tb in range(TB):
            xt = p_x.tile([P, KO, P], F32R, tag="x")
            nc.sync.dma_start(xt, xT[tb])
            pkv = psum.tile([P, 2 * FKV], F32, tag="mm", bufs=4)
            for ko in range(KO):
                nc.tensor.matmul(pkv, xt[:, ko, :], wkv[:, ko, :],
                                 start=(ko == 0), stop=(ko == KO - 1))
            nc.scalar.copy(V[:, tb, :], pkv[:, FKV:])
            ktmp = p_tmp.tile([P, FKV], F32, tag="qk")
            cos_b = cos_sb[:, tb, None, :].to_broadcast([P, HKV, HD // 2])
            sin_b = sin_sb[:, tb, None, :].to_broadcast([P, HKV, HD // 2])
            _rope_rms(nc, pkv[:, 0:FKV], ktmp, p_tmp, p_st, cos_b, sin_b, eps_sb, HKV)
            for h in range(HKV):
                ptr = psum.tile([P, P], F32, tag="small", bufs=2)
                nc.tensor.transpose(ptr, ktmp[:, h * HD:(h + 1) * HD], ident)
                nc.scalar.copy(KT[:, h, tb * P:(tb + 1) * P], ptr)

        # ---------------- Q halves + attention ------------------------------
        for half in range(2):
            q0 = half * HQH
            QT = p_qt.tile([P, HQH, S], F32R, tag="qt")   # [hd, h, tok]
            wq = p_w.tile([P, KO, 2 * FKV], F32R, tag="w")
            nc.sync.dma_start(wq, wqT[:, :, q0 * HD:(q0 + HQH) * HD])
            for tb in range(TB):
                xt = p_x.tile([P, KO, P], F32R, tag="x")
                nc.sync.dma_start(xt, xT[tb])
                pq = psum.tile([P, HQH * HD], F32, tag="mm", bufs=4)
                for ko in range(KO):
                    nc.tensor.matmul(pq, xt[:, ko, :], wq[:, ko, :],
                                     start=(ko == 0), stop=(ko == KO - 1))
                qtmp = p_tmp.tile([P, HQH * HD], F32, tag="qk")
                cos_b = cos_sb[:, tb, None, :].to_broadcast([P, HQH, HD // 2])
                sin_b = sin_sb[:, tb, None, :].to_broadcast([P, HQH, HD // 2])
                _rope_rms(nc, pq, qtmp, p_tmp, p_st, cos_b, sin_b, eps_sb, HQH)
                for h in range(HQH):
                    ptr = psum.tile([P, P], F32, tag="small", bufs=2)
                    nc.tensor.transpose(ptr, qtmp[:, h * HD:(h + 1) * HD], ident)
                    nc.scalar.copy(QT[:, h, tb * P:(tb + 1) * P], ptr)

            if half == 1:
                # prefetch wo rows for heads 0-3 into the freed weight slot;
                # the DMA overlaps attention of the second half
                woT0 = p_w.tile([P, KO, 2 * FKV], F32R, tag="w")
                nc.sync.dma_start(
                    woT0.rearrange("p ko f -> p (ko f)").rearrange(
                        "p (h d) -> p h d", h=HQH),
                    woTr[:, 0:HQH, :])

            # attention for these 4 heads; kv head == half
            kvh = half
            for h in range(HQH):
                for qc in range(QC):
                    kb_hi = 4 * (qc + 1) if mode == "causal" else KB
                    po = psum.tile([P, 512], F32, tag="po", bufs=2)
                    pd = psum.tile([1, 512], F32, tag="small", bufs=2)
                    for kb in range(kb_hi):
                        j0 = max(0, kb * P - qc * 512) if mode == "causal" else 0
                        pss = psum.tile([P, 512], F32, tag="mm", bufs=4)
                        nc.tensor.matmul(
                            pss[:, j0:], KT[:, kvh, kb * P:(kb + 1) * P],
                            QT[:, h, qc * 512 + j0:(qc + 1) * 512],
                            start=True, stop=True)
                        if mode == "general":
                            mt = p_pb.tile([P, 512], F32, tag="mt")
                            nc.sync.dma_start(
                                mt, maskTd[kb * P:(kb + 1) * P, qc * 512:(qc + 1) * 512])
                            nc.vector.tensor_add(pss, pss, mt)
                        pbt = p_pb.tile([P, 512], F32R, tag="pb")
                        nc.scalar.activation(pbt[:, j0:], pss[:, j0:], Exp, scale=SCALE)
                        if mode == "causal" and kb >= 4 * qc:
                            nc.vector.tensor_mul(pbt[:, j0:j0 + P],
                                                 pbt[:, j0:j0 + P], dmask_sb)
                        first, last = (kb == 0), (kb == kb_hi - 1)
                        nc.tensor.matmul(po[:, j0:],
                                         V[:, kb, kvh * HD:(kvh + 1) * HD],
                                         pbt[:, j0:], start=first, stop=last)
                        nc.tensor.matmul(pd[:, j0:], ones_sb, pbt[:, j0:],
                                         start=first, stop=last)
                    rec = p_st.tile([1, 512], F32, tag="rec", bufs=2)
                    nc.vector.reciprocal(rec, pd)
                    recb = p_pb.tile([P, 512], F32, tag="recb", bufs=2)
                    nc.gpsimd.partition_broadcast(recb, rec)
                    ot = p_ot.tile([P, 512], F32R, tag="ot")
                    nc.vector.tensor_mul(ot, po, recb)
                    nc.sync.dma_start(oTd[:, q0 + h, qc * 512:(qc + 1) * 512], ot)

        p_pb.release()
        p_qt.release()
        p_st.release()
        p_tmp.release()
        p_x.release()
        kvp.release()

        # ---------------- O projection --------------------------------------
        p_wo = tc.alloc_tile_pool(name="p_wo", bufs=1)
        p_ox = tc.alloc_tile_pool(name="p_ox", bufs=2)
        woT1 = p_wo.tile([P, HQH, D], F32R)
        nc.sync.dma_start(woT1, woTr[:, HQH:, :])
        woT0v = woT0.rearrange("p ko f -> p (ko f)").rearrange(
            "p (h d) -> p h d", h=HQH)
        for sc in range(QC):
            oT = p_ox.tile([P, HQ, 512], F32R, tag="ox")
            nc.sync.dma_start(oT, oTd[:, :, sc * 512:(sc + 1) * 512])
            for t4 in range(4):
                tb = sc * 4 + t4
                for dc in range(D // 512):
                    pso = psum.tile([P, 512], F32, tag="mm", bufs=4)
                    for ko in range(HQ):
                        w_sl = (woT0v[:, ko, dc * 512:(dc + 1) * 512] if ko < HQH
                                else woT1[:, ko - HQH, dc * 512:(dc + 1) * 512])
                        nc.tensor.matmul(pso, oT[:, ko, t4 * P:(t4 + 1) * P], w_sl,
                                         start=(ko == 0), stop=(ko == HQ - 1))
                    osb = p_ot.tile([P, 512], F32, tag="osb")
                    nc.scalar.copy(osb, pso)
                    nc.sync.dma_start(out_p[tb * P:(tb + 1) * P, dc * 512:(dc + 1) * 512], osb)
        p_ox.release()
        p_wo.release()
        p_w.release()
        p_ot.release()
        psum.release()
        dramp.release()
        singles.release()

    nc.compile()
    return nc


_NC_CACHE = {}


def _get_nc(mode):
    if mode not in _NC_CACHE:
        _NC_CACHE[mode] = _build(mode)
    return _NC_CACHE[mode]


def _detect_mode(mask):
    il, jl = np.tril_indices(SEQ)
    if np.all(mask == 0.0):
        return "zero"
    iu, ju = np.triu_indices(SEQ, k=1)
    if np.all(mask[il, jl] == 0.0) and np.all(mask[iu, ju] <= -1e30):
        return "causal"
    return "general"


def _prep_in_maps(x, wq, wk, wv, wo, cos, sin, mask, mode):
    f = np.float32
    maps = []
    cosf = np.ascontiguousarray(cos, dtype=f)
    sinf = np.ascontiguousarray(sin, dtype=f)
    if mode == "causal":
        i = np.arange(P)
        dmask = (i[:, None] <= i[None, :]).astype(f)   # keep where k <= q
    if mode == "general":
        maskT = np.maximum(mask.T.astype(np.float64) / SCALE, -3e38).astype(f)
        maskT = np.ascontiguousarray(maskT)
    def shuf_x(xb):
        # [S, D] -> [tb, p, ko, s]; element [tb,p,ko,s] = x[tb*P+s, ko*P+p]
        return np.ascontiguousarray(
            xb.reshape(TB, P, KO, P).transpose(0, 3, 2, 1), dtype=f)

    def shuf_w(wT):
        # [D, F] -> [p, ko, f]
        F = wT.shape[1]
        return np.ascontiguousarray(
            wT.reshape(KO, P, F).transpose(1, 0, 2), dtype=f)

    cosf = np.ascontiguousarray(cosf.reshape(TB, P, HD // 2).transpose(1, 0, 2))
    sinf = np.ascontiguousarray(sinf.reshape(TB, P, HD // 2).transpose(1, 0, 2))
    for c in range(NCORES):
        b, g = divmod(c, GROUPS)
        m = {
            "xT": shuf_x(x[b]),
            "wqT": shuf_w(wq[FQ * g:FQ * (g + 1)].T),
            "wkT": shuf_w(wk[FKV * g:FKV * (g + 1)].T),
            "wvT": shuf_w(wv[FKV * g:FKV * (g + 1)].T),
            "woT": np.ascontiguousarray(wo[:, FQ * g:FQ * (g + 1)].T, dtype=f),
            "cosd": cosf,
            "sind": sinf,
            "onesd": np.ones((P, 1), dtype=f),
        }
        if mode == "causal":
            m["dmask"] = dmask
        if mode == "general":
            m["maskT"] = maskT
        maps.append(m)
    return maps


def _numpy_fallback(x, wq, wk, wv, wo, cos, sin, mask):
    """Exact-semantics numpy fallback (only for unexpected inputs)."""
    b, s, _ = x.shape
    xq = (x @ wq.T).reshape(b, s, N_HEADS, HD)
    xk = (x @ wk.T).reshape(b, s, N_KV, HD)
    xv = (x @ wv.T).reshape(b, s, N_KV, HD)

    def rope(t):
        tf = t.reshape(*t.shape[:-1], HD // 2, 2)
        e, o = tf[..., 0], tf[..., 1]
        c = cos[None, :, None, :]
        sn = sin[None, :, None, :]
        return np.stack([e * c - o * sn, e * sn + o * c], axis=-1).reshape(t.shape)

    def rms(t):
        return t * (1.0 / np.sqrt(np.mean(t * t, axis=-1, keepdims=True) + EPS))

    xq = rms(rope(xq))
    xk = rms(rope(xk))
    q = xq.transpose(0, 2, 1, 3).reshape(b, N_KV, N_HEADS // N_KV, s, HD)
    k = xk.transpose(0, 2, 1, 3)
    v = xv.transpose(0, 2, 1, 3)
    scores = np.einsum("bgrqd,bgkd->bgrqk", q, k) * SCALE + mask[None, None, None]
    scores -= scores.max(axis=-1, keepdims=True)
    p = np.exp(scores)
    p /= p.sum(axis=-1, keepdims=True)
    out = np.einsum("bgrqk,bgkd->bgrqd", p, v)
    out = out.reshape(b, N_HEADS, s, HD).transpose(0, 2, 1, 3).reshape(b, s, N_HEADS * HD)
    return (out @ wo.T).astype(np.float32)


def kernel(x, wq, wk, wv, wo, cos, sin, mask, start_pos=0, **_unused):
    x = np.asarray(x, dtype=np.float32)
    wq = np.asarray(wq, dtype=np.float32)
    wk = np.asarray(wk, dtype=np.float32)
    wv = np.asarray(wv, dtype=np.float32)
    wo = np.asarray(wo, dtype=np.float32)
    cos = np.asarray(cos, dtype=np.float32)
    sin = np.asarray(sin, dtype=np.float32)
    mask = np.asarray(mask, dtype=np.float32)
    if int(np.asarray(start_pos)) != 0:
        return _numpy_fallback(x, wq, wk, wv, wo, cos, sin, mask)

    mode = _detect_mode(mask)
    nc = _get_nc(mode)
    in_maps = _prep_in_maps(x, wq, wk, wv, wo, cos, sin, mask, mode)
    res = bass_utils.run_bass_kernel_spmd(nc, in_maps, core_ids=list(range(NCORES)))
    out = np.empty((BSZ, S, D), dtype=np.float32)
    for b in range(BSZ):
        acc = res.results[b * GROUPS]["out_p"].astype(np.float32).copy()
        for g in range(1, GROUPS):
            acc += res.results[b * GROUPS + g]["out_p"]
        out[b] = acc
    return out


if __name__ == "__main__":
    # quick self-exercise with random inputs (not the reference distribution)
    rng = np.random.default_rng(0)
    x = rng.standard_normal((BSZ, SEQ, DIM), dtype=np.float32)
    wq = (rng.standard_normal((DIM, DIM), dtype=np.float32) * 0.02)
    wk = (rng.standard_normal((N_KV * HD, DIM), dtype=np.float32) * 0.02)
    wv = (rng.standard_normal((N_KV * HD, DIM), dtype=np.float32) * 0.02)
    wo = (rng.standard_normal((DIM, DIM), dtype=np.float32) * 0.02)
    inv = 1.0 / (500000.0 ** (np.arange(0, HD, 2) / HD))
    ang = np.arange(SEQ)[:, None] * inv[None, :]
    cos, sin = np.cos(ang).astype(np.float32), np.sin(ang).astype(np.float32)
    mask = np.where(np.tril(np.ones((SEQ, SEQ), bool)), 0.0,
                    np.finfo(np.float32).min).astype(np.float32)
    got = kernel(x=x, wq=wq, wk=wk, wv=wv, wo=wo, cos=cos, sin=sin,
                 mask=mask, start_pos=0)
    want = _numpy_fallback(x, wq, wk, wv, wo, cos, sin, mask)
    err = np.linalg.norm(got - want) / np.linalg.norm(want)
    print("rel err vs numpy:", err)
